# revision 1
# baseline (speedup 1.0000x reference)
"""BinaryConv2d on 8 TRN2 NeuronCores.

Problem: x (32,256,56,56) f32, weights (256,256,3,3) f32.
  out = conv2d(x, sign(weights)), NCHW/OIHW, stride 1, VALID -> (32,256,54,54).

Strategy (data-parallel): 4 images per core, weights (tiny, binarized)
replicated. On each core the conv is computed as 18 PSUM-accumulating
matmuls per output tile: 9 kernel taps x 2 input-channel tiles of 128.
  lhsT[c,o] = sign(W)[o,c,kh,kw]          (stationary, fp16, exact +-1)
  rhs[c, 9x54] = x[c, y0+kh : y0+kh+9, kw : kw+OW]  (moving, fp16)
  psum[o, 486] += lhsT.T @ rhs            (fp32 accumulation)
Free dim N = 9*54 = 486 <= 512 (one PSUM bank). 54 = 6 blocks of 9 rows.
fp16 (not bf16): binarized weights are exact either way, and fp16's 10
mantissa bits cut the x-rounding error ~8x at identical PE throughput.

Startup engineering: x input DMAs ride the sync-engine HWDGE queues and
weights + output DMAs ride the scalar-engine queues so they move in
parallel; x is split into row chunks and w into per-(ct,ot) quarters so
the first accumulation group's deps land early; a short dummy-matmul
warmup keeps the PE busy from the end of the framework preamble until
the first chunks land, so the HAM clock-gate is already at 8/8 when the
real stream starts. The final output block is split in two so its PSUM
drain + output DMA overlap the closing matmuls.
"""

import os
import sys

import numpy as np

for _p in ("/opt/trn_rl_repo", "/root/.axon_site/_ro/trn_rl_repo"):
    if os.path.isdir(_p) and _p not in sys.path:
        sys.path.insert(0, _p)

import concourse.bacc as bacc
import concourse.mybir as mybir
from concourse import tile
from concourse.bass_utils import run_bass_kernel_spmd

N_CORES = 8
B, C, H, W = 32, 256, 56, 56
O, KH, KW = 256, 3, 3
OH, OW = H - KH + 1, W - KW + 1  # 54, 54
BPC = B // N_CORES  # images per core
CT = C // 128  # input-channel tiles
OT = O // 128  # output-channel tiles
YR = 9  # output rows per matmul block
YB = OH // YR  # 6 blocks
NF = YR * OW  # 486 free dim
NKK = KH * KW  # 9 taps
# x row chunks: yb block j reads input rows [9j, 9j+11). Chunk boundaries
# chosen so the first matmuls' data lands as early as possible.
XCHUNKS = (0, 11, 20, 29, 56)
WARMUP_MM = 8  # dummy matmuls to lift the PE HAM clock-gate during load;
# sized to keep the PE continuously busy from the end of the framework
# preamble (~7.8us) until the first input chunks land (~10.8us), so the
# HAM activity window never sees an idle gap before the real stream.
# (Early DMA delivery runs at only ~150GB/s aggregate while the DGE
# descriptor path ramps, so the first chunks cannot usefully land sooner;
# finer-grained first chunks were measured to only move the stall.)

_NC_CACHE = {}


def _build():
    nc = bacc.Bacc("TRN2", target_bir_lowering=False, debug=False)
    fp16 = mybir.dt.float16
    x_d = nc.dram_tensor("x", [BPC, C, H, W], fp16, kind="ExternalInput")
    w_d = nc.dram_tensor("w", [CT, OT, 128, NKK, 128], fp16, kind="ExternalInput")
    out_d = nc.dram_tensor(
        "out", [BPC, O, OH, OW], mybir.dt.float32, kind="ExternalOutput"
    )
    x_ap = x_d.ap()
    w_ap = w_d.ap()
    out_flat = out_d.ap().rearrange("b o h w -> b o (h w)")

    with tile.TileContext(nc) as tc:
        with (
            tc.tile_pool(name="wpool", bufs=1) as wpool,
            tc.tile_pool(name="xpool", bufs=2) as xpool,
            tc.tile_pool(name="opool", bufs=4) as opool,
            tc.tile_pool(name="pspool", bufs=6, space="PSUM") as pspool,
            tc.tile_pool(name="pswarm", bufs=1, space="PSUM") as pswarm,
        ):
            # PE warmup: HAM un-throttles after ~3.4us of sustained PE work.
            # Burn dummy matmuls on a zero tile while the input DMAs land so
            # the real matmul stream starts at 2.4 GHz instead of 1.2.
            # (A dependency-free warmup on an uninitialized tile would start
            # ~1.4us earlier still, but the simulator rejects the read.)
            zt = wpool.tile([128, 512], fp16, tag="warm")
            nc.gpsimd.memset(zt[:], 0.0)
            wps = pswarm.tile([128, 512], mybir.dt.float32)
            for _ in range(WARMUP_MM):
                nc.tensor.matmul(wps[:], zt[:, :128], zt[:], start=True, stop=True)

            def x_load(n):
                """Load image n (n>=1): the plain tile plus a one-column-
                shifted copy. SBUF matmul reads are 4-byte granular, so the
                kw=1 tap's 2-byte (one fp16 column) offset costs +8ns per
                matmul; kw=1 reads the shifted copy at an aligned offset.
                The copy rides the mostly-idle Vector engine with ~45us of
                prefetch slack."""
                xts, xos = [], []
                for ct in range(CT):
                    xt = xpool.tile([128, H, W], fp16, tag=f"x{ct}")
                    xts.append(xt)
                for lo, hi in zip(XCHUNKS, XCHUNKS[1:]):  # top chunks first
                    for ct in range(CT):
                        nc.sync.dma_start(
                            xts[ct][:, lo:hi], x_ap[n, ct * 128 : (ct + 1) * 128, lo:hi]
                        )
                for ct in range(CT):
                    xo = xpool.tile([128, H, W], fp16, tag=f"xo{ct}")
                    nc.vector.tensor_copy(xo[:, :, 0 : W - 1], xts[ct][:, :, 1:W])
                    xos.append(xo)
                return xts, xos

            # x rides the sync-engine HWDGE queues, weights + outputs ride
            # the scalar-engine queues, so input streams move in parallel
            # (they share the core's HBM bandwidth either way). Image 0's
            # chunks are issued ct0-first to match the ct0-first matmul
            # order below; deadlines checked against the ~165GB/s early
            # aggregate DMA rate.
            x0ts = [
                xpool.tile([128, H, W], fp16, tag="x0", name="x0t_first"),
                xpool.tile([128, H, W], fp16, tag="x1", name="x1t_first"),
            ]

            def x0_chunk(ct, ci):
                lo, hi = XCHUNKS[ci], XCHUNKS[ci + 1]
                nc.sync.dma_start(
                    x0ts[ct][:, lo:hi], x_ap[0, ct * 128 : (ct + 1) * 128, lo:hi]
                )

            for ct, ci in ((0, 0), (0, 1), (0, 2), (1, 0), (0, 3), (1, 1), (1, 2), (1, 3)):
                x0_chunk(ct, ci)
            w_sb = wpool.tile([128, CT, OT, NKK, 128], fp16)
            for ot in range(OT):  # first group is ot=0: load its halves first
                for ct in range(CT):
                    nc.scalar.dma_start(w_sb[:, ct, ot], w_ap[ct, ot])

            def emit_group(xts, n, ot, y0, rows, xos=None):
                ps = pspool.tile([128, rows * OW], mybir.dt.float32, tag="ps")
                k = 0
                for ct in range(CT):
                    for kh in range(KH):
                        for kw in range(KW):
                            if kw == 1 and xos is not None:
                                rhs = xos[ct][:, y0 + kh : y0 + kh + rows, 0:OW]
                            else:
                                rhs = xts[ct][:, y0 + kh : y0 + kh + rows, kw : kw + OW]
                            nc.tensor.matmul(
                                ps[:],
                                w_sb[:, ct, ot, kh * KW + kw, :],
                                rhs,
                                start=(k == 0),
                                stop=(k == KH * KW * CT - 1),
                            )
                            k += 1
                ob = opool.tile([128, rows * OW], mybir.dt.float32, tag="ob")
                nc.vector.tensor_copy(ob[:], ps[:])
                nc.scalar.dma_start(
                    out_flat[
                        n, ot * 128 : (ot + 1) * 128, y0 * OW : (y0 + rows) * OW
                    ],
                    ob[:],
                )

            # First three blocks of image 0: run all ct=0 taps of all three
            # before any ct=1 tap (interleaved PSUM accumulation groups on
            # three banks). The ct=0 chunks land first on the ramping DMA
            # queues; this pushes the ct=1 dependency deadline ~5.5us later,
            # making the startup schedule feasible at the early DMA rate and
            # removing the measured stall at matmul #9.
            pre = [
                pspool.tile([128, NF], mybir.dt.float32, tag="ps", name=f"ps_pre{i}")
                for i in range(3)
            ]
            for ct in range(CT):
                for yb in range(3):
                    y0 = yb * YR
                    for kh in range(KH):
                        for kw in range(KW):
                            nc.tensor.matmul(
                                pre[yb][:],
                                w_sb[:, ct, 0, kh * KW + kw, :],
                                x0ts[ct][:, y0 + kh : y0 + kh + YR, kw : kw + OW],
                                start=(ct == 0 and kh == 0 and kw == 0),
                                stop=(ct == CT - 1 and kh == KH - 1 and kw == KW - 1),
                            )
            for yb in range(3):
                ob = opool.tile(
                    [128, NF], mybir.dt.float32, tag="ob", name=f"ob_pre{yb}"
                )
                nc.vector.tensor_copy(ob[:], pre[yb][:])
                nc.scalar.dma_start(
                    out_flat[0, 0:128, yb * YR * OW : (yb + 1) * YR * OW], ob[:]
                )

            for n in range(BPC):
                if n == 0:
                    xts, xos = x0ts, None  # startup-critical: unaligned kw=1
                else:
                    xts, xos = x_load(n)
                for ot in range(OT):
                    for yb in range(YB):
                        if n == 0 and ot == 0 and yb < 3:
                            continue  # emitted above
                        last = n == BPC - 1 and ot == OT - 1 and yb == YB - 1
                        if not last:
                            emit_group(xts, n, ot, yb * YR, YR, xos)
                        else:
                            # Split the final block by rows so its PSUM drain +
                            # output DMA overlap the closing matmuls.
                            emit_group(xts, n, ot, yb * YR, 5, xos)
                            emit_group(xts, n, ot, yb * YR + 5, 4, xos)
    nc.compile()
    return nc


def get_nc():
    if "nc" not in _NC_CACHE:
        _NC_CACHE["nc"] = _build()
    return _NC_CACHE["nc"]


def prep_inputs(x, weights):
    """Full f32 inputs -> per-core in_maps (fp16)."""
    x = np.ascontiguousarray(np.asarray(x, dtype=np.float32))
    weights = np.asarray(weights, dtype=np.float32)
    qw = np.sign(weights).astype(np.float32)  # [O, I, KH, KW]
    w6 = qw.reshape(OT, 128, CT, 128, KH, KW)  # [ot, o, ct, c, kh, kw]
    wt = np.transpose(w6, (2, 0, 3, 4, 5, 1))  # [ct, ot, c, kh, kw, o]
    w5 = np.ascontiguousarray(wt).reshape(CT, OT, 128, NKK, 128).astype(np.float16)
    x_f16 = x.reshape(N_CORES, BPC, C, H, W).astype(np.float16)
    return [{"x": x_f16[i], "w": w5} for i in range(N_CORES)]


def run_spmd(in_maps, **kwargs):
    nc = get_nc()
    return run_bass_kernel_spmd(nc, in_maps, list(range(N_CORES)), **kwargs)


def kernel(x, weights):
    in_maps = prep_inputs(x, weights)
    res = run_spmd(in_maps)
    out = np.concatenate(
        [np.asarray(res.results[i]["out"]) for i in range(N_CORES)], axis=0
    )
    return np.ascontiguousarray(out.astype(np.float32))



# revision 2
# speedup vs baseline: 1.0028x; 1.0028x over previous
"""BinaryConv2d on 8 TRN2 NeuronCores — 1-D Winograd F(2,3) along H, fp16.

Problem: x (32,256,56,56) f32, weights (256,256,3,3) f32.
  out = conv2d(x, sign(weights)), NCHW/OIHW, stride 1, VALID -> (32,256,54,54).

Data-parallel: 4 images per core, weights replicated. The 3 kh-taps are
replaced by 4 Winograd row-planes shared by 2 output rows each, cutting
PE work to 2/3 of direct conv (the kw taps stay direct, so rhs slices
are the baseline's proven strided [128, rows, 54] shape):

  input transform (DVE, fp16, per channel plane ct, per row-pair i):
    T1[i] = x[2i+1] + x[2i+2]      T2[i] = x[2i+2] - x[2i+1]
    T0[i] = x[2i]   - x[2i+2]      T3[i] = x[2i+1] - x[2i+3]
  matmul (PE, fp16, per plane p: 6 accumulating matmuls = 3 kw x 2 ct):
    M_p[o, i, j] = sum_{c,kw} U_p[o, c, kw] * T_p[c, i, j+kw]
    with U = G @ sign(w): entries are multiples of 0.5 -> fp16-exact.
  output transform (Scalar copies M1,M2 out of PSUM; DVE combines):
    out[2i]   = M0 + M1 + M2       out[2i+1] = M1 - M2 - M3

27 row-pairs i in blocks of 9; psum free dim 9*54 = 486. The 4 M-planes
of a block live in 4 PSUM banks; tag rings of 2 give full double
buffering (8 banks). Matmul plane order p1,p2,p0,p3 (transforms emitted
in the same order) lets the M1/M2 drains start while p0/p3 stream.

Scheduling: image 0 is emitted ib-major with its first block split
(2+3+4 row-pairs) so the earliest matmuls need only the first x chunks;
the ot=1 weight half and late chunks land in the shadow of running
blocks. Transforms for image n+1 are spliced between the last blocks of
image n — DVE is in-order, so this keeps them from stalling the drain
ops that recycle PSUM banks. The final block is split 5/4 with p3
closed before p0 to shorten the drain tail.

Measured rel err ~3.7e-4 (fp16-level, Winograd rounding included).
PE roofline: 576 matmuls x 486 rows x 0.417ns = 117us vs 175us direct.
"""

import os
import sys

import numpy as np

for _p in ("/opt/trn_rl_repo", "/root/.axon_site/_ro/trn_rl_repo"):
    if os.path.isdir(_p) and _p not in sys.path:
        sys.path.insert(0, _p)

import concourse.bacc as bacc
import concourse.mybir as mybir
from concourse import tile
from concourse.bass_utils import run_bass_kernel_spmd

N_CORES = 8
B, C, H, W = 32, 256, 56, 56
O, KH, KW = 256, 3, 3
OH, OW = H - KH + 1, W - KW + 1  # 54, 54
BPC = B // N_CORES  # images per core
CT = C // 128  # input-channel planes
OT = O // 128  # output-channel tiles
NP = 4  # winograd row planes
NI = OH // 2  # 27 row-pair tiles
IB = 9  # row-pairs per matmul block
NB = NI // IB  # 3 blocks
HWF = H * W
XCHUNKS = (0, 7, 13, 20, 29, 38, 56)
# transform spans: span (i0,i1) needs x rows up to 2*(i1-1)+3.
SPANS0 = ((0, 2), (2, 5), (5, 9), (9, 13), (13, 18), (18, NI))  # image 0
SPANS = ((0, 9), (9, 14), (14, NI))  # steady state
# image-0 blocks, ib-major: (ot, i0, nrow)
BLOCKS0 = (
    (0, 0, 2), (0, 2, 3), (0, 5, 4), (1, 0, 9),
    (0, 9, 9), (1, 9, 9), (0, 18, 9), (1, 18, 9),
)
WARMUP_MM = 8  # dummy matmuls lifting the PE HAM clock-gate during load

_NC_CACHE = {}


def _build():
    nc = bacc.Bacc("TRN2", target_bir_lowering=False, debug=False)
    fp16 = mybir.dt.float16
    f32 = mybir.dt.float32
    ADD = mybir.AluOpType.add
    SUB = mybir.AluOpType.subtract
    COPY = mybir.ActivationFunctionType.Copy
    # (plane, row offset a, op, row offset b): T_p[i] = x[2i+a] op x[2i+b];
    # emitted in the matmul plane order so p1/p2 unlock first.
    TFS = ((1, 1, ADD, 2), (2, 2, SUB, 1), (0, 0, SUB, 2), (3, 1, SUB, 3))
    x_d = nc.dram_tensor("x", [BPC, 128, CT, HWF], fp16, kind="ExternalInput")
    w_d = nc.dram_tensor("w", [128, NP, KW, OT, CT, 128], fp16, kind="ExternalInput")
    out_d = nc.dram_tensor(
        "out", [BPC, O, OH, OW], mybir.dt.float32, kind="ExternalOutput"
    )
    x_ap = x_d.ap()
    w_ap = w_d.ap()
    out_flat = out_d.ap().rearrange("b o h w -> b o (h w)")

    with tile.TileContext(nc) as tc:
        with (
            tc.tile_pool(name="wpool", bufs=1) as wpool,
            tc.tile_pool(name="xpool", bufs=2) as xpool,
            tc.tile_pool(name="tpool", bufs=2) as tpool,
            tc.tile_pool(name="opool", bufs=2) as opool,
            tc.tile_pool(name="pspool", bufs=2, space="PSUM") as pspool,
        ):
            # PE warmup while the first input chunks land. Rides generation
            # 0 of the ps1 tag ring; real groups rotate onto other banks.
            zt = wpool.tile([128, IB * OW], fp16, tag="warm")
            nc.gpsimd.memset(zt[:], 0.0)
            wps = pspool.tile([128, IB * OW], f32, tag="ps1", name="ps_warm")
            for _ in range(WARMUP_MM):
                nc.tensor.matmul(wps[:], zt[:, :128], zt[:], start=True, stop=True)

            def x_load(n):
                xt = xpool.tile([128, CT, H, W], fp16, tag="x")
                xtf = xt[:].rearrange("q c h w -> q c (h w)")
                for lo, hi in zip(XCHUNKS, XCHUNKS[1:]):
                    nc.sync.dma_start(
                        xtf[:, :, lo * W : hi * W], x_ap[n, :, :, lo * W : hi * W]
                    )
                return xt

            def new_t():
                return tpool.tile([128, CT, NP, NI, W], fp16, tag="T", name="tt")

            def tf_span(tt, xt, i0, i1):
                ni = i1 - i0
                for p, a, op, b in TFS:
                    for ct in range(CT):
                        nc.vector.tensor_tensor(
                            tt[:, ct, p, i0:i1, :],
                            xt[:, ct, a + 2 * i0 : a + 2 * i0 + 2 * ni - 1 : 2, :],
                            xt[:, ct, b + 2 * i0 : b + 2 * i0 + 2 * ni - 1 : 2, :],
                            op,
                        )

            def emit_block(tt, n, ot, i0, nrow, p_order=(1, 2, 0, 3)):
                """nrow row-pairs starting at row-pair i0 -> 2*nrow out rows."""
                ps = {}
                for p in p_order:
                    ps[p] = pspool.tile(
                        [128, nrow, OW], f32, tag=f"ps{p}", name=f"psb{p}"
                    )
                    psf = ps[p][:].rearrange("q r w -> q (r w)")
                    k = 0
                    for kw in range(KW):
                        for ct in range(CT):
                            nc.tensor.matmul(
                                psf,
                                w_sb[:, p, kw, ot, ct],
                                tt[:, ct, p, i0 : i0 + nrow, kw : kw + OW],
                                start=(k == 0),
                                stop=(k == KW * CT - 1),
                            )
                            k += 1
                s1 = opool.tile([128, nrow, OW], f32, tag="s1")
                s2 = opool.tile([128, nrow, OW], f32, tag="s2")
                nc.scalar.activation(s1[:], ps[1][:], COPY)
                nc.scalar.activation(s2[:], ps[2][:], COPY)
                bm = opool.tile([128, nrow, OW], f32, tag="bm")
                am = opool.tile([128, nrow, OW], f32, tag="am")
                nc.vector.tensor_tensor(bm[:], s1[:], s2[:], SUB)  # M1-M2
                nc.vector.tensor_tensor(am[:], ps[0][:], s1[:], ADD)  # M0+M1
                ob = opool.tile([128, 2 * nrow, OW], f32, tag="ob")
                nc.vector.tensor_tensor(ob[:, 0::2, :], am[:], s2[:], ADD)
                nc.vector.tensor_tensor(ob[:, 1::2, :], bm[:], ps[3][:], SUB)
                nc.scalar.dma_start(
                    out_flat[
                        n,
                        ot * 128 : (ot + 1) * 128,
                        2 * i0 * OW : 2 * (i0 + nrow) * OW,
                    ],
                    ob[:],
                )

            # Weights ride the scalar queue: ot=0 half first (the first
            # blocks only need it), ot=1 in the shadow of running blocks.
            x0 = x_load(0)
            w_sb = wpool.tile([128, NP, KW, OT, CT, 128], fp16)
            nc.scalar.dma_start(w_sb[:, :, :, 0], w_ap[:, :, :, 0])
            nc.scalar.dma_start(w_sb[:, :, :, 1], w_ap[:, :, :, 1])
            t_cur = new_t()
            for sp in SPANS0:
                tf_span(t_cur, x0, *sp)

            for n in range(BPC):
                if n == 0:
                    blocks = BLOCKS0
                else:
                    blocks = [(ot, ib * IB, IB) for ot in range(OT) for ib in range(NB)]
                if n < BPC - 1:
                    x_next = x_load(n + 1)
                    t_next = new_t()
                else:
                    x_next, t_next = None, None
                    # split the final block: drain + output DMA overlap the
                    # closing matmuls; p3 closes before p0 in the tail.
                    blocks = blocks[:-1] + [(1, 18, 5)]
                nsp = len(blocks) - len(SPANS)
                for j, (ot, i0, nrow) in enumerate(blocks):
                    emit_block(t_cur, n, ot, i0, nrow)
                    if t_next is not None and j >= nsp:
                        tf_span(t_next, x_next, *SPANS[j - nsp])
                if n == BPC - 1:
                    emit_block(t_cur, n, 1, 23, 4, p_order=(1, 2, 3, 0))
                t_cur = t_next
    nc.compile()
    return nc


def get_nc():
    if "nc" not in _NC_CACHE:
        _NC_CACHE["nc"] = _build()
    return _NC_CACHE["nc"]


def prep_inputs(x, weights):
    """Full f32 inputs -> per-core in_maps (fp16, Winograd weights)."""
    x = np.ascontiguousarray(np.asarray(x, dtype=np.float32))
    weights = np.asarray(weights, dtype=np.float32)
    qw = np.sign(weights).astype(np.float32)  # [O, I, KH, KW]
    G = np.array(
        [[1, 0, 0], [0.5, 0.5, 0.5], [0.5, -0.5, 0.5], [0, 0, 1]], np.float32
    )
    U = np.einsum("pk,oikw->poiw", G, qw)  # multiples of 0.5 -> fp16 exact
    U6 = U.reshape(NP, OT, 128, CT, 128, KW)
    wt = np.transpose(U6, (4, 0, 5, 1, 3, 2))  # [c, p, kw, ot, ct, o]
    w_np = np.ascontiguousarray(wt).astype(np.float16)

    x6 = x.reshape(N_CORES, BPC, CT, 128, H, W)
    x6 = np.transpose(x6, (0, 1, 3, 2, 4, 5))  # [core, n, c, ct, h, w]
    x_np = np.ascontiguousarray(x6).astype(np.float16).reshape(
        N_CORES, BPC, 128, CT, HWF
    )
    return [{"x": x_np[i], "w": w_np} for i in range(N_CORES)]


def run_spmd(in_maps, **kwargs):
    nc = get_nc()
    return run_bass_kernel_spmd(nc, in_maps, list(range(N_CORES)), **kwargs)


def kernel(x, weights):
    in_maps = prep_inputs(x, weights)
    res = run_spmd(in_maps)
    out = np.concatenate(
        [np.asarray(res.results[i]["out"]) for i in range(N_CORES)], axis=0
    )
    return np.ascontiguousarray(out.astype(np.float32))
